# revision 50
# baseline (speedup 1.0000x reference)
"""Causal self-attention (GQA + RoPE) Trainium2 Bass kernel, 8-core SPMD.

Problem shapes (hardcoded): B=2, S=2048, D=1024, NH=16 q-heads, KVH=4
kv-heads, HD=64, RoPE base 10000, fp32 I/O.

Sharding (batch x kv-group): core c -> batch b = c//4, kv-group g = c%4.
Each kv-group owns one kv head and its 4 q heads (GQA repeat=4), so the
whole causal attention for those heads is local to the core. Each core
computes the partial output projection y_g @ Wo[g-block]; the host sums
the 4 partials per batch ("gather/unshard").

Pipelined over 4 seq-chunks of 512: x chunk DMA -> qkv projection ->
RoPE -> v transpose -> attention q-tile (over all earlier key chunks)
-> output projection, interleaved so the PE queue never runs dry while
softmax-division or rope chains drain (PE idle >3.4us would re-throttle
the clock to 1.2 GHz). A burst of dummy matmuls on a zeroed tile warms
the PE clock while the input DMAs land. All inputs ship bf16 and
host-pre-arranged into the exact SBUF layouts (contiguous DMA lines).

Everything on-chip is kept transposed ([head_dim, seq]) so QK^T and PV
need no transposes: scores_T[k, q] = k_T.T @ q_T with the head dim as
the PE contraction axis, and y_T[d, q] = v_chunk.T @ expP_T. QK^T
matmuls are split into 64x64 array tiles (auto tile_position) so up to
4 run concurrently despite the 64-deep contraction. RoPE's rotate_half
uses no DMAs: a sign-folded sin table indexed by SOURCE row + DVE ops
whose output partition base is shifted by 32 (inputs must share a base
partition; outputs may differ). Both q head-pairs live pair-major in
one tile so each rope multiply covers them in a single [*, 2, 512] op.
Softmax runs along the PARTITION axis for free: v is augmented with a
ones column, so PV accumulation drops the denominator into row 64 of
y_aug; division = ACT copies of the denominators -> one DVE reciprocal
(reciprocal_approx_fast must NOT read PSUM) -> gpsimd
partition_broadcast -> one PSUM-direct multiply per head writing the
persistent Y tile. Causality is handled at 128-granularity: score
matmuls and exp start at the first unmasked query column, fully-masked
key chunks are skipped, and one 128x128 triangular mask multiply
handles each diagonal block. Output partials are bf16, summed on host.
"""
import numpy as np
from contextlib import ExitStack

import ml_dtypes

import concourse.bass as bass
import concourse.tile as tile
from concourse import bacc, mybir
from concourse.bass_utils import run_bass_kernel_spmd

F32 = mybir.dt.float32
BF16 = mybir.dt.bfloat16
FP8 = mybir.dt.float8e4
AF = mybir.ActivationFunctionType

B, S, D = 2, 2048, 1024
NH, KVH, HD = 16, 4, 64
N_CORES = 8
SCALE = HD ** -0.5  # 0.125
NCH = 4           # seq chunks
CW = S // NCH     # 512 chunk width

BF = ml_dtypes.bfloat16

_CACHE = {}


def _rope_tables():
    half = HD // 2
    inv_freq = (1.0 / (10000.0 ** (np.arange(half, dtype=np.float32) / half))
                ).astype(np.float32)
    t = np.arange(S, dtype=np.float32)
    freqs = np.outer(t, inv_freq).astype(np.float32)      # [S, 32]
    emb = np.concatenate([freqs, freqs], axis=1)          # [S, 64]
    cos_T = np.cos(emb).T.astype(np.float32)              # [64, S]
    sin_T = np.sin(emb).T.astype(np.float32)
    # sin table indexed by the rotate_half SOURCE row: out[d] for d<32 reads
    # src s=d+32 and needs -sin -> rows 32:64 carry -sin; out[d>=32] reads
    # s=d-32 and needs +sin -> rows 0:32 carry +sin (freq halves identical).
    sin_s = sin_T.copy()
    sin_s[half:] *= -1.0
    cos4 = np.tile(cos_T, (2, 1))                         # [128, S] (2 heads)
    sin4 = np.tile(sin_s, (2, 1))
    return cos4.astype(BF), sin4.astype(BF)


def _build_kernel():
    nc = bacc.Bacc("TRN2", target_bir_lowering=False, debug=False,
                   num_devices=N_CORES)

    # all inputs ship host-pre-arranged into the exact SBUF layouts, so every
    # input DMA is contiguous per partition (2-8KB lines, no tiny packets)
    xT_ap = nc.dram_tensor("xT", [128, NCH, 8, CW], BF16,
                           kind="ExternalInput").ap()
    wq_ap = nc.dram_tensor("wq", [128, 8, 256], BF16,
                           kind="ExternalInput").ap()
    wkv_ap = nc.dram_tensor("wkv", [128, 8, 128], BF16,
                            kind="ExternalInput").ap()
    wo_ap = nc.dram_tensor("wo", [128, 2, 1024], BF16,
                           kind="ExternalInput").ap()
    out_ap = nc.dram_tensor("out", [S, D], BF16, kind="ExternalOutput").ap()

    cos4_np, sin4_np = _rope_tables()
    cos4_d = nc.inline_tensor(cos4_np, name="cos4").ap()
    sin4_d = nc.inline_tensor(sin4_np, name="sin4").ap()
    tri_np = (np.arange(128)[None, :] >= np.arange(128)[:, None]
              ).astype(BF)                                # [k, q] keep q>=k
    tri_d = nc.inline_tensor(tri_np, name="tri").ap()

    with tile.TileContext(nc) as tc, ExitStack() as top:
        # ---- persistent sbuf tiles -------------------------------------
        const = top.enter_context(tc.tile_pool(name="const", bufs=1))
        # pair-major doubled rope tables: [:, 0:S] and [:, S:2S] identical,
        # so one [128, 2, 512]-shaped multiply covers both q head-pairs
        cos8 = const.tile([128, 2 * S], BF16, tag="cos8")
        sin8 = const.tile([128, 2 * S], BF16, tag="sin8")
        tri = const.tile([128, 128], BF16, tag="tri")

        wpool = top.enter_context(tc.tile_pool(name="w", bufs=1))
        wq_sb = wpool.tile([128, 8 * 256], BF16, tag="wq")
        wkv_sb = wpool.tile([128, 8 * 128], BF16, tag="wkv")
        wo_sb = wpool.tile([128, 2 * 1024], BF16, tag="wo")

        act = top.enter_context(tc.tile_pool(name="acts", bufs=1))
        # both q head-pairs in one pair-major tile: qp_c[:, pair*S + s]
        qp_c = act.tile([128, 2 * S], BF16, tag="qpc")
        kk = act.tile([128, S], BF16, tag="kk")
        # v stationary padded to 128 cols (64 v + ones + pad) to keep FWL on
        v_all = act.tile([128, 16 * 128], BF16, tag="v_all")
        Y = [act.tile([128, S], BF16, tag=f"Y{i}", name=f"Y{i}") for i in range(2)]

        xpool = top.enter_context(tc.tile_pool(name="xT", bufs=4))
        rawp = top.enter_context(tc.tile_pool(name="raw", bufs=2))
        shp = top.enter_context(tc.tile_pool(name="shift", bufs=2))

        # PSUM: proj 1 bank + scores 4 banks + y/outproj pool 3 banks = 8.
        # 3 y-banks let the next pair's PV start while the previous pair's
        # division still holds its bank (ypool also serves outproj's po).
        pjp = top.enter_context(tc.tile_pool(name="pj", bufs=1, space="PSUM"))
        spool = top.enter_context(tc.tile_pool(name="sc", bufs=2, space="PSUM"))
        ypool = top.enter_context(tc.tile_pool(name="yps", bufs=3, space="PSUM"))

        epool = top.enter_context(tc.tile_pool(name="ex", bufs=3))
        dpool = top.enter_context(tc.tile_pool(name="div", bufs=2))
        osb = top.enter_context(tc.tile_pool(name="osb", bufs=3))

        # ---- PE warm-up: ~3.5us of dummy matmuls on a zeroed tile while
        # the input DMAs land, so the HAM un-throttles the PE clock to
        # 2.4 GHz before the first real matmul issues ------------------
        warm = wpool.tile([128, 512], BF16, tag="warm")
        warm_ps = ypool.tile([128, 512], F32, tag="y", name="warmps")
        nc.vector.memset(warm[:], 0.0)
        nc.vector.memset(v_all[:], 0.0)
        for _w in range(8):
            nc.tensor.matmul(warm_ps[:], warm[:, 0:128], warm[:],
                             start=True, stop=True)

        # ---- upfront DMAs, arrival-ordered -----------------------------
        xch = [xpool.tile([128, 8, CW], BF16, tag="x", name=f"x{j}")
               for j in range(NCH)]
        # only ~5 early dispatches per ring flow freely -- later entries wait
        # 15-20us on cross-ring DMA-slot recycling. Everything needed before
        # ~20us must sit in a free-flowing slot on one of the two rings.
        nc.gpsimd.dma_start(wkv_sb[:].rearrange("p (kc m) -> p kc m", kc=8),
                            wkv_ap[:])
        nc.gpsimd.dma_start(wq_sb[:].rearrange("p (kc m) -> p kc m", kc=8),
                            wq_ap[:])
        # x0 in two halves so proj(0)'s first kc-chains start off the first
        nc.sync.dma_start(xch[0][:, 0:4], xT_ap[:, 0, 0:4])
        nc.sync.dma_start(xch[0][:, 4:8], xT_ap[:, 0, 4:8])
        nc.gpsimd.dma_start(xch[1][:], xT_ap[:, 1])
        nc.sync.dma_start(xch[2][:], xT_ap[:, 2])
        nc.gpsimd.dma_start(xch[3][:], xT_ap[:, 3])
        nc.gpsimd.dma_start(cos8[:, 0:S], cos4_d[:])
        nc.sync.dma_start(sin8[:, 0:S], sin4_d[:])
        nc.sync.dma_start(tri[:], tri_d[:])
        nc.gpsimd.dma_start(wo_sb[:].rearrange("p (c n) -> p c n", c=2),
                            wo_ap[:])
        nc.vector.tensor_copy(cos8[:, S:2 * S], cos8[:, 0:S])
        nc.vector.tensor_copy(sin8[:, S:2 * S], sin8[:, 0:S])
        # ones column per v slot via a strided memset - a DMA here would
        # wait on the v_all memset and poison the ring/semaphore pool with
        # a late producer (false deps via shared completion semaphores)
        ones_cols = v_all[:].rearrange("p (s c) -> p s c", c=128)[:, :, 64]
        nc.vector.memset(ones_cols, 1.0)

        def proj(j):
            """projections for seq chunk j -> kvraw/qraw tiles (bf16)."""
            jsl = slice(j * CW, (j + 1) * CW)
            with nc.named_scope("proj"):
                kv_ps = pjp.tile([128, CW], F32, tag="pj", name=f"kvps{j}")
                q0_ps = pjp.tile([128, CW], F32, tag="pj", name=f"q0ps{j}")
                for kc in range(8):
                    nc.tensor.matmul(
                        kv_ps[:], wkv_sb[:, kc * 128:(kc + 1) * 128],
                        xch[j][:, kc, :], start=(kc == 0), stop=(kc == 7))
                    nc.tensor.matmul(
                        q0_ps[:], wq_sb[:, kc * 256:kc * 256 + 128],
                        xch[j][:, kc, :], start=(kc == 0), stop=(kc == 7))
                kvraw = rawp.tile([128, CW], BF16, tag="kvraw", name=f"kvraw{j}")
                qraw_c = rawp.tile([128, 2 * CW], BF16, tag="qrawc",
                                   name=f"qrawc{j}")
                nc.vector.tensor_copy(kvraw[:], kv_ps[:])
                q1_ps = pjp.tile([128, CW], F32, tag="pj", name=f"q1ps{j}")
                for kc in range(8):
                    nc.tensor.matmul(
                        q1_ps[:], wq_sb[:, kc * 256 + 128:kc * 256 + 256],
                        xch[j][:, kc, :], start=(kc == 0), stop=(kc == 7))
                # q psum evictions on the (otherwise idle) scalar engine
                nc.scalar.copy(qraw_c[:, 0:CW], q0_ps[:])
                nc.scalar.copy(qraw_c[:, CW:2 * CW], q1_ps[:])
            with nc.named_scope("rope"):
                # rotate_half via 32-aligned partition-offset DVE reads (no
                # DMAs: shift DMAs head-of-line-block the rings); both q
                # head-pairs processed by one [*, 2, 512]-shaped op each
                ksh = shp.tile([64, CW], BF16, tag="ksh", name=f"ksh{j}")
                nc.vector.tensor_mul(ksh[0:32, :], kvraw[32:64, :],
                                     sin8[32:64, jsl])
                nc.vector.tensor_mul(ksh[32:64, :], kvraw[0:32, :],
                                     sin8[0:32, jsl])
                nc.vector.tensor_mul(kk[0:64, jsl], kvraw[0:64, :],
                                     cos8[0:64, jsl])
                nc.vector.tensor_add(kk[0:64, jsl], kk[0:64, jsl], ksh[:])
                nc.vector.tensor_copy(kk[64:128, jsl], kk[0:64, jsl])

                qsh = shp.tile([128, 2 * CW], BF16, tag="qsh", name=f"qsh{j}")
                qraw_v = qraw_c[:].rearrange("p (two q) -> p two q", two=2)
                qsh_v = qsh[:].rearrange("p (two q) -> p two q", two=2)
                qp_v = qp_c[:].rearrange("p (two s) -> p two s", two=2)
                cos_v = cos8[:].rearrange("p (two s) -> p two s", two=2)
                sin_v = sin8[:].rearrange("p (two s) -> p two s", two=2)
                for h in range(2):
                    hb = h * 64
                    nc.vector.tensor_mul(qsh_v[hb:hb + 32],
                                         qraw_v[hb + 32:hb + 64],
                                         sin_v[hb + 32:hb + 64, :, jsl])
                    nc.vector.tensor_mul(qsh_v[hb + 32:hb + 64],
                                         qraw_v[hb:hb + 32],
                                         sin_v[hb:hb + 32, :, jsl])
                nc.vector.tensor_mul(qp_v[:, :, jsl], qraw_v[:],
                                     cos_v[:, :, jsl])
                nc.vector.tensor_add(qp_v[:, :, jsl], qp_v[:, :, jsl],
                                     qsh_v[:])
            with nc.named_scope("vprep"):
                # v rows (partitions 64:128 of kvraw) -> [seq, 64] tiles
                for st in range(4 * j, 4 * j + 4):
                    nc.sync.dma_start_transpose(
                        v_all[:, st * 128:st * 128 + 64],
                        kvraw[64:128, (st - 4 * j) * 128:(st - 4 * j + 1) * 128])

        def attn_pair(qt, pair):
            nkc = 4 * qt + 4
            qsl = slice(qt * CW, (qt + 1) * CW)
            with nc.named_scope("attn"):
                y_ps = [ypool.tile([128, CW], F32, tag="y",
                                   name=f"y{pair}{qt}{_h}") for _h in range(2)]
                for G in range(nkc // 2):
                    sc = [spool.tile([128, 1024], F32, tag="sc",
                                     name=f"sc{pair}{qt}{G}{_h}")
                          for _h in range(2)]
                    offs = []
                    for ci in range(2):
                        kc = 2 * G + ci
                        jd = kc - 4 * qt
                        off = jd * 128 if 0 <= jd < 4 else 0
                        offs.append(off)
                        # 4-way 64x64 PE tiling: (hl, kb) -> tiles
                        # T0/T2/T8/T10 run concurrently (tile_position is
                        # auto-derived from the operand base partitions)
                        for hl in range(2):  # adjacent row groups
                            hb = hl * 64
                            for kb in range(2):  # key sub-block -> psum half
                                nc.tensor.matmul(
                                    sc[hl][kb * 64:kb * 64 + 64,
                                           ci * 512 + off:(ci + 1) * 512],
                                    kk[hb:hb + 64, kc * 128 + kb * 64:
                                       kc * 128 + kb * 64 + 64],
                                    qp_c[hb:hb + 64,
                                         pair * S + qt * CW + off:
                                         pair * S + (qt + 1) * CW],
                                    start=True, stop=True)
                    ex = [epool.tile([128, 1024], BF16, tag="ex",
                                     name=f"ex{pair}{qt}{G}{_h}")
                          for _h in range(2)]
                    e0 = offs[0]
                    for hl in range(2):
                        nc.scalar.activation(ex[hl][:, e0:1024],
                                             sc[hl][:, e0:1024],
                                             AF.Exp, scale=SCALE)
                    for ci in range(2):
                        kc = 2 * G + ci
                        off = offs[ci]
                        jd = kc - 4 * qt
                        for hl in range(2):
                            if 0 <= jd < 4:  # diagonal chunk: mask
                                mslice = ex[hl][:, ci * 512 + off:
                                                ci * 512 + off + 128]
                                nc.vector.tensor_mul(mslice, mslice, tri[:])
                            nc.tensor.matmul(
                                y_ps[hl][:, off:512],
                                v_all[:, kc * 128:(kc + 1) * 128],
                                ex[hl][:, ci * 512 + off:(ci + 1) * 512],
                                start=(kc == 0), stop=(kc == nkc - 1))
                # softmax division: both heads' denominators batched into one
                # reciprocal; the normalize multiply reads y straight from
                # PSUM and writes the persistent Y tile (no staging copy/DMA)
                dn = dpool.tile([1, 1024], F32, tag="dn",
                                name=f"dn{pair}{qt}")
                for hl in range(2):
                    # on ACT: it idles in exactly these windows, while the
                    # vector queue is clumped with rope + division muls
                    nc.scalar.copy(dn[:, hl * 512:hl * 512 + 512],
                                   y_ps[hl][64:65, :])
                recf = dpool.tile([1, 1024], F32, tag="recf",
                                  name=f"rec{pair}{qt}")
                nc.vector.reciprocal_approx_fast(recf[:], dn[:])
                for hl in range(2):
                    bc = dpool.tile([64, 512], F32, tag="bc",
                                    name=f"bc{pair}{qt}{hl}")
                    nc.gpsimd.partition_broadcast(
                        bc[:], recf[:, hl * 512:hl * 512 + 512])
                    nc.vector.tensor_mul(
                        Y[pair][hl * 64:hl * 64 + 64, qsl],
                        y_ps[hl][0:64, :], bc[:])

        oev = [0]

        def outproj(qt):
            with nc.named_scope("outproj"):
                for st in range(4 * qt, 4 * qt + 4):
                    ot = osb.tile([128, 1024], BF16, tag="ot")
                    for nt in range(2):
                        po = ypool.tile([128, 512], F32, tag="y",
                                        name=f"po{st}{nt}")
                        for cc in range(2):
                            nc.tensor.matmul(
                                po[:],
                                Y[cc][:, st * 128:(st + 1) * 128],
                                wo_sb[:, cc * 1024 + nt * 512:
                                      cc * 1024 + (nt + 1) * 512],
                                start=(cc == 0), stop=(cc == 1))
                        nc.vector.tensor_copy(
                            ot[:, nt * 512:(nt + 1) * 512], po[:])
                    nc.sync.dma_start(
                        out_ap[st * 128:(st + 1) * 128, :], ot[:])

        # proj(j) is issued 1-2 chunks ahead of the attention that needs it
        # (rope is pure-DVE now, so interleaving costs no DMA-ring blocking,
        # and fine-grained interleave keeps the vector queue from serializing
        # a bulk rope burst in front of the attention mask/divide ops).
        # outproj(j) is issued only after attn_pair(j+1, 0) so the PE queue
        # has attention matmuls while q-tile j's division chains drain.
        proj(0)
        proj(1)
        attn_pair(0, 0)
        proj(2)
        attn_pair(0, 1)
        proj(3)
        attn_pair(1, 0)
        for j in range(1, NCH):
            outproj(j - 1)
            attn_pair(j, 1)
            if j + 1 < NCH:
                attn_pair(j + 1, 0)
        outproj(NCH - 1)

    nc.compile()
    return nc


def _shard_inputs(x, Wq, Wk, Wv, Wo):
    # host-side pre-arrangement into SBUF layouts (contiguous DMA lines):
    # xT[p, j, kc, s] = x[b].T[kc*128+p, j*512+s]
    in_maps = []
    xb = []
    for b in range(B):
        xT = np.ascontiguousarray(x[b].T).astype(BF)          # [D, S]
        xb.append(np.ascontiguousarray(
            xT.reshape(8, 128, NCH, CW).transpose(1, 2, 0, 3)))
    for c in range(N_CORES):
        b, g = divmod(c, 4)
        wq = Wq[:, g * 256:(g + 1) * 256].astype(BF)          # [1024, 256]
        wkv = np.concatenate(
            [Wk[:, g * 64:(g + 1) * 64], Wv[:, g * 64:(g + 1) * 64]],
            axis=1).astype(BF)                                # [1024, 128]
        wo = Wo[g * 256:(g + 1) * 256, :].astype(BF)          # [256, 1024]
        in_maps.append({
            "xT": xb[b],
            "wq": np.ascontiguousarray(
                wq.reshape(8, 128, 256).transpose(1, 0, 2)),
            "wkv": np.ascontiguousarray(
                wkv.reshape(8, 128, 128).transpose(1, 0, 2)),
            "wo": np.ascontiguousarray(
                wo.reshape(2, 128, 1024).transpose(1, 0, 2)),
        })
    return in_maps


def kernel(x, Wq, Wk, Wv, Wo):
    x = np.asarray(x, dtype=np.float32)
    Wq = np.asarray(Wq, dtype=np.float32)
    Wk = np.asarray(Wk, dtype=np.float32)
    Wv = np.asarray(Wv, dtype=np.float32)
    Wo = np.asarray(Wo, dtype=np.float32)
    assert x.shape == (B, S, D), x.shape

    if "nc" not in _CACHE:
        _CACHE["nc"] = _build_kernel()
    nc = _CACHE["nc"]

    in_maps = _shard_inputs(x, Wq, Wk, Wv, Wo)
    res = run_bass_kernel_spmd(nc, in_maps, list(range(N_CORES)))

    out = np.zeros((B, S, D), dtype=np.float32)
    for c in range(N_CORES):
        out[c // 4] += np.asarray(res.results[c]["out"], dtype=np.float32)
    return out



# revision 55
# speedup vs baseline: 1.1723x; 1.1723x over previous
"""Causal self-attention (GQA + RoPE) Trainium2 Bass kernel, 8-core SPMD.

Problem shapes (hardcoded): B=2, S=2048, D=1024, NH=16 q-heads, KVH=4
kv-heads, HD=64, RoPE base 10000, fp32 I/O.

Sharding (batch x kv-group): core c -> batch b = c//4, kv-group g = c%4.
Each kv-group owns one kv head and its 4 q heads (GQA repeat=4), so the
whole causal attention for those heads is local to the core. Each core
computes the partial output projection y_g @ Wo[g-block]; the host sums
the 4 partials per batch ("gather/unshard").

Pipelined over 4 seq-chunks of 512: x chunk DMA -> qkv projection ->
RoPE -> v transpose -> attention q-tile (over all earlier key chunks)
-> output projection, interleaved so the PE queue never runs dry while
softmax-division or rope chains drain (PE idle >3.4us would re-throttle
the clock to 1.2 GHz). A burst of dummy matmuls on a zeroed tile warms
the PE clock while the input DMAs land. All inputs ship bf16 and
host-pre-arranged into the exact SBUF layouts (contiguous DMA lines).

Everything on-chip is kept transposed ([head_dim, seq]) so QK^T and PV
need no transposes: scores_T[k, q] = k_T.T @ q_T with the head dim as
the PE contraction axis, and y_T[d, q] = v_chunk.T @ expP_T. QK^T
matmuls are split into 64x64 array tiles (auto tile_position) so up to
4 run concurrently despite the 64-deep contraction. RoPE's rotate_half
uses no DMAs: a sign-folded sin table indexed by SOURCE row + DVE ops
whose output partition base is shifted by 32 (inputs must share a base
partition; outputs may differ). Both q head-pairs live pair-major in
one tile so each rope multiply covers them in a single [*, 2, 512] op.
Softmax runs along the PARTITION axis for free: v is augmented with a
ones column, so PV accumulation drops the denominator into row 64 of
y_aug; division = ACT copies of the denominators -> one DVE reciprocal
(reciprocal_approx_fast must NOT read PSUM) -> gpsimd
partition_broadcast -> one PSUM-direct multiply per head writing the
persistent Y tile. Causality is handled at 128-granularity: score
matmuls and exp start at the first unmasked query column, fully-masked
key chunks are skipped, and one 128x128 triangular mask multiply
handles each diagonal block. Output partials are bf16, summed on host.
"""
import numpy as np
from contextlib import ExitStack

import ml_dtypes

import concourse.bass as bass
import concourse.tile as tile
from concourse import bacc, mybir
from concourse.bass_utils import run_bass_kernel_spmd

F32 = mybir.dt.float32
BF16 = mybir.dt.bfloat16
FP8 = mybir.dt.float8e4
AF = mybir.ActivationFunctionType

B, S, D = 2, 2048, 1024
NH, KVH, HD = 16, 4, 64
N_CORES = 8
SCALE = HD ** -0.5  # 0.125
NCH = 4           # seq chunks
CW = S // NCH     # 512 chunk width

BF = ml_dtypes.bfloat16

_CACHE = {}


def _rope_tables():
    half = HD // 2
    inv_freq = (1.0 / (10000.0 ** (np.arange(half, dtype=np.float32) / half))
                ).astype(np.float32)
    t = np.arange(S, dtype=np.float32)
    freqs = np.outer(t, inv_freq).astype(np.float32)      # [S, 32]
    emb = np.concatenate([freqs, freqs], axis=1)          # [S, 64]
    cos_T = np.cos(emb).T.astype(np.float32)              # [64, S]
    sin_T = np.sin(emb).T.astype(np.float32)
    # sin table indexed by the rotate_half SOURCE row: out[d] for d<32 reads
    # src s=d+32 and needs -sin -> rows 32:64 carry -sin; out[d>=32] reads
    # s=d-32 and needs +sin -> rows 0:32 carry +sin (freq halves identical).
    sin_s = sin_T.copy()
    sin_s[half:] *= -1.0
    cos4 = np.tile(cos_T, (2, 1))                         # [128, S] (2 heads)
    sin4 = np.tile(sin_s, (2, 1))
    return cos4.astype(BF), sin4.astype(BF)


def _build_kernel():
    nc = bacc.Bacc("TRN2", target_bir_lowering=False, debug=False,
                   num_devices=N_CORES)

    # all inputs ship host-pre-arranged into the exact SBUF layouts, so every
    # input DMA is contiguous per partition (2-8KB lines, no tiny packets)
    xT_ap = nc.dram_tensor("xT", [128, NCH, 8, CW], BF16,
                           kind="ExternalInput").ap()
    wq_ap = nc.dram_tensor("wq", [128, 8, 256], BF16,
                           kind="ExternalInput").ap()
    wkv_ap = nc.dram_tensor("wkv", [128, 8, 128], BF16,
                            kind="ExternalInput").ap()
    wo_ap = nc.dram_tensor("wo", [128, 2, 1024], BF16,
                           kind="ExternalInput").ap()
    out_ap = nc.dram_tensor("out", [S, D], BF16, kind="ExternalOutput").ap()

    cos4_np, sin4_np = _rope_tables()
    cos4_d = nc.inline_tensor(cos4_np, name="cos4").ap()
    sin4_d = nc.inline_tensor(sin4_np, name="sin4").ap()
    tri_np = (np.arange(128)[None, :] >= np.arange(128)[:, None]
              ).astype(BF)                                # [k, q] keep q>=k
    tri_d = nc.inline_tensor(tri_np, name="tri").ap()

    with tile.TileContext(nc) as tc, ExitStack() as top:
        # ---- persistent sbuf tiles -------------------------------------
        const = top.enter_context(tc.tile_pool(name="const", bufs=1))
        # pair-major doubled rope tables: [:, 0:S] and [:, S:2S] identical,
        # so one [128, 2, 512]-shaped multiply covers both q head-pairs
        cos8 = const.tile([128, 2 * S], BF16, tag="cos8")
        sin8 = const.tile([128, 2 * S], BF16, tag="sin8")
        tri = const.tile([128, 128], BF16, tag="tri")

        wpool = top.enter_context(tc.tile_pool(name="w", bufs=1))
        wq_sb = wpool.tile([128, 8 * 256], BF16, tag="wq")
        wkv_sb = wpool.tile([128, 8 * 128], BF16, tag="wkv")
        wo_sb = wpool.tile([128, 2 * 1024], BF16, tag="wo")

        act = top.enter_context(tc.tile_pool(name="acts", bufs=1))
        # both q head-pairs in one pair-major tile: qp_c[:, pair*S + s]
        qp_c = act.tile([128, 2 * S], BF16, tag="qpc")
        kk = act.tile([128, S], BF16, tag="kk")
        # v stationary padded to 128 cols (64 v + ones + pad) to keep FWL on
        v_all = act.tile([128, 16 * 128], BF16, tag="v_all")
        Y = [act.tile([128, S], BF16, tag=f"Y{i}", name=f"Y{i}") for i in range(2)]

        xpool = top.enter_context(tc.tile_pool(name="xT", bufs=4))
        rawp = top.enter_context(tc.tile_pool(name="raw", bufs=2))
        shp = top.enter_context(tc.tile_pool(name="shift", bufs=2))

        # PSUM: pj/outproj 2 banks + scores 4 banks + y 2 banks = 8
        pjp = top.enter_context(tc.tile_pool(name="pj", bufs=2, space="PSUM"))
        spool = top.enter_context(tc.tile_pool(name="sc", bufs=2, space="PSUM"))
        ypool = top.enter_context(tc.tile_pool(name="yps", bufs=2, space="PSUM"))

        epool = top.enter_context(tc.tile_pool(name="ex", bufs=3))
        dpool = top.enter_context(tc.tile_pool(name="div", bufs=2))
        osb = top.enter_context(tc.tile_pool(name="osb", bufs=3))

        # ---- PE warm-up: ~3.5us of dummy matmuls on a zeroed tile while
        # the input DMAs land, so the HAM un-throttles the PE clock to
        # 2.4 GHz before the first real matmul issues ------------------
        warm = wpool.tile([128, 512], BF16, tag="warm")
        warm_ps = ypool.tile([128, 512], F32, tag="y", name="warmps")
        nc.vector.memset(warm[:], 0.0)
        nc.vector.memset(v_all[:], 0.0)
        for _w in range(8):
            nc.tensor.matmul(warm_ps[:], warm[:, 0:128], warm[:],
                             start=True, stop=True)

        # ---- upfront DMAs, arrival-ordered -----------------------------
        xch = [xpool.tile([128, 8, CW], BF16, tag="x", name=f"x{j}")
               for j in range(NCH)]
        # only ~5 early dispatches per ring flow freely -- later entries wait
        # 15-20us on cross-ring DMA-slot recycling. Everything needed before
        # ~20us must sit in a free-flowing slot on one of the two rings.
        nc.gpsimd.dma_start(wkv_sb[:].rearrange("p (kc m) -> p kc m", kc=8),
                            wkv_ap[:])
        nc.gpsimd.dma_start(wq_sb[:].rearrange("p (kc m) -> p kc m", kc=8),
                            wq_ap[:])
        nc.sync.dma_start(xch[0][:], xT_ap[:, 0])
        nc.gpsimd.dma_start(xch[1][:], xT_ap[:, 1])
        nc.sync.dma_start(xch[2][:], xT_ap[:, 2])
        nc.gpsimd.dma_start(xch[3][:], xT_ap[:, 3])
        nc.gpsimd.dma_start(cos8[:, 0:S], cos4_d[:])
        nc.sync.dma_start(sin8[:, 0:S], sin4_d[:])
        nc.sync.dma_start(tri[:], tri_d[:])
        nc.gpsimd.dma_start(wo_sb[:].rearrange("p (c n) -> p c n", c=2),
                            wo_ap[:])
        nc.vector.tensor_copy(cos8[:, S:2 * S], cos8[:, 0:S])
        nc.vector.tensor_copy(sin8[:, S:2 * S], sin8[:, 0:S])
        # ALL 64 pad columns of each v slot are ones (not just col 64): the
        # PV matmul then replicates the softmax denominator across y_ps rows
        # 64:128 for free (the stationary is 128 wide regardless), so the
        # division needs no partition_broadcast. memset, not DMA: a DMA here
        # would poison the ring/semaphore pool with a late producer.
        ones_cols = v_all[:].rearrange("p (s c) -> p s c", c=128)[:, :, 64:128]
        nc.vector.memset(ones_cols, 1.0)

        def proj(j):
            """projections for seq chunk j -> kvraw/qraw tiles (bf16)."""
            jsl = slice(j * CW, (j + 1) * CW)
            with nc.named_scope("proj"):
                kv_ps = pjp.tile([128, CW], F32, tag="pj", name=f"kvps{j}")
                q0_ps = pjp.tile([128, CW], F32, tag="pj", name=f"q0ps{j}")
                for kc in range(8):
                    nc.tensor.matmul(
                        kv_ps[:], wkv_sb[:, kc * 128:(kc + 1) * 128],
                        xch[j][:, kc, :], start=(kc == 0), stop=(kc == 7))
                    nc.tensor.matmul(
                        q0_ps[:], wq_sb[:, kc * 256:kc * 256 + 128],
                        xch[j][:, kc, :], start=(kc == 0), stop=(kc == 7))
                kvraw = rawp.tile([128, CW], BF16, tag="kvraw", name=f"kvraw{j}")
                qraw_c = rawp.tile([128, 2 * CW], BF16, tag="qrawc",
                                   name=f"qrawc{j}")
                nc.vector.tensor_copy(kvraw[:], kv_ps[:])
                q1_ps = pjp.tile([128, CW], F32, tag="pj", name=f"q1ps{j}")
                for kc in range(8):
                    nc.tensor.matmul(
                        q1_ps[:], wq_sb[:, kc * 256 + 128:kc * 256 + 256],
                        xch[j][:, kc, :], start=(kc == 0), stop=(kc == 7))
                # q psum evictions on the (otherwise idle) scalar engine
                nc.scalar.copy(qraw_c[:, 0:CW], q0_ps[:])
                nc.scalar.copy(qraw_c[:, CW:2 * CW], q1_ps[:])
            with nc.named_scope("rope"):
                # rotate_half via 32-aligned partition-offset DVE reads (no
                # DMAs: shift DMAs head-of-line-block the rings); both q
                # head-pairs processed by one [*, 2, 512]-shaped op each
                ksh = shp.tile([64, CW], BF16, tag="ksh", name=f"ksh{j}")
                nc.vector.tensor_mul(ksh[0:32, :], kvraw[32:64, :],
                                     sin8[32:64, jsl])
                nc.vector.tensor_mul(ksh[32:64, :], kvraw[0:32, :],
                                     sin8[0:32, jsl])
                nc.vector.tensor_mul(kk[0:64, jsl], kvraw[0:64, :],
                                     cos8[0:64, jsl])
                nc.vector.tensor_add(kk[0:64, jsl], kk[0:64, jsl], ksh[:])
                nc.vector.tensor_copy(kk[64:128, jsl], kk[0:64, jsl])

                qsh = shp.tile([128, 2 * CW], BF16, tag="qsh", name=f"qsh{j}")
                qraw_v = qraw_c[:].rearrange("p (two q) -> p two q", two=2)
                qsh_v = qsh[:].rearrange("p (two q) -> p two q", two=2)
                qp_v = qp_c[:].rearrange("p (two s) -> p two s", two=2)
                cos_v = cos8[:].rearrange("p (two s) -> p two s", two=2)
                sin_v = sin8[:].rearrange("p (two s) -> p two s", two=2)
                for h in range(2):
                    hb = h * 64
                    nc.vector.tensor_mul(qsh_v[hb:hb + 32],
                                         qraw_v[hb + 32:hb + 64],
                                         sin_v[hb + 32:hb + 64, :, jsl])
                    nc.vector.tensor_mul(qsh_v[hb + 32:hb + 64],
                                         qraw_v[hb:hb + 32],
                                         sin_v[hb:hb + 32, :, jsl])
                nc.vector.tensor_mul(qp_v[:, :, jsl], qraw_v[:],
                                     cos_v[:, :, jsl])
                nc.vector.tensor_add(qp_v[:, :, jsl], qp_v[:, :, jsl],
                                     qsh_v[:])
            with nc.named_scope("vprep"):
                # v rows (partitions 64:128 of kvraw) -> [seq, 64] tiles
                for st in range(4 * j, 4 * j + 4):
                    nc.sync.dma_start_transpose(
                        v_all[:, st * 128:st * 128 + 64],
                        kvraw[64:128, (st - 4 * j) * 128:(st - 4 * j + 1) * 128])

        def attn_pair(qt, pair):
            nkc = 4 * qt + 4
            qsl = slice(qt * CW, (qt + 1) * CW)
            with nc.named_scope("attn"):
                y_ps = [ypool.tile([128, CW], F32, tag="y",
                                   name=f"y{pair}{qt}{_h}") for _h in range(2)]
                for G in range(nkc // 2):
                    sc = [spool.tile([128, 1024], F32, tag="sc",
                                     name=f"sc{pair}{qt}{G}{_h}")
                          for _h in range(2)]
                    offs = []
                    for ci in range(2):
                        kc = 2 * G + ci
                        jd = kc - 4 * qt
                        off = jd * 128 if 0 <= jd < 4 else 0
                        offs.append(off)
                        # 4-way 64x64 PE tiling: (hl, kb) -> tiles
                        # T0/T2/T8/T10 run concurrently (tile_position is
                        # auto-derived from the operand base partitions)
                        for hl in range(2):  # adjacent row groups
                            hb = hl * 64
                            for kb in range(2):  # key sub-block -> psum half
                                nc.tensor.matmul(
                                    sc[hl][kb * 64:kb * 64 + 64,
                                           ci * 512 + off:(ci + 1) * 512],
                                    kk[hb:hb + 64, kc * 128 + kb * 64:
                                       kc * 128 + kb * 64 + 64],
                                    qp_c[hb:hb + 64,
                                         pair * S + qt * CW + off:
                                         pair * S + (qt + 1) * CW],
                                    start=True, stop=True)
                    ex = [epool.tile([128, 1024], BF16, tag="ex",
                                     name=f"ex{pair}{qt}{G}{_h}")
                          for _h in range(2)]
                    e0 = offs[0]
                    for hl in range(2):
                        nc.scalar.activation(ex[hl][:, e0:1024],
                                             sc[hl][:, e0:1024],
                                             AF.Exp, scale=SCALE)
                    for ci in range(2):
                        kc = 2 * G + ci
                        off = offs[ci]
                        jd = kc - 4 * qt
                        for hl in range(2):
                            if 0 <= jd < 4:  # diagonal chunk: mask
                                mslice = ex[hl][:, ci * 512 + off:
                                                ci * 512 + off + 128]
                                nc.vector.tensor_mul(mslice, mslice, tri[:])
                            nc.tensor.matmul(
                                y_ps[hl][:, off:512],
                                v_all[:, kc * 128:(kc + 1) * 128],
                                ex[hl][:, ci * 512 + off:(ci + 1) * 512],
                                start=(kc == 0), stop=(kc == nkc - 1))
                # softmax division: y_ps rows 64:128 already hold the
                # denominator replicated 64x (ones pad columns in v), so the
                # chain is copy -> full-width reciprocal -> PSUM-direct mul,
                # with no partition_broadcast (saves ~2.4us of latency per
                # pair, freeing the y PSUM bank for the next pair sooner)
                for hl in range(2):
                    dsb = dpool.tile([64, 512], F32, tag="dsb",
                                     name=f"dsb{pair}{qt}{hl}")
                    # on ACT: it idles in exactly these windows, while the
                    # vector queue is clumped with rope + division muls
                    nc.scalar.copy(dsb[:], y_ps[hl][64:128, :])
                    rec = dpool.tile([64, 512], F32, tag="recf",
                                     name=f"rec{pair}{qt}{hl}")
                    nc.vector.reciprocal_approx_fast(rec[:], dsb[:])
                    nc.vector.tensor_mul(
                        Y[pair][hl * 64:hl * 64 + 64, qsl],
                        y_ps[hl][0:64, :], rec[:])

        oev = [0]

        def outproj(qt):
            with nc.named_scope("outproj"):
                for st in range(4 * qt, 4 * qt + 4):
                    ot = osb.tile([128, 1024], BF16, tag="ot")
                    for nt in range(2):
                        po = pjp.tile([128, 512], F32, tag="pj",
                                      name=f"po{st}{nt}")
                        for cc in range(2):
                            nc.tensor.matmul(
                                po[:],
                                Y[cc][:, st * 128:(st + 1) * 128],
                                wo_sb[:, cc * 1024 + nt * 512:
                                      cc * 1024 + (nt + 1) * 512],
                                start=(cc == 0), stop=(cc == 1))
                        nc.vector.tensor_copy(
                            ot[:, nt * 512:(nt + 1) * 512], po[:])
                    nc.sync.dma_start(
                        out_ap[st * 128:(st + 1) * 128, :], ot[:])

        # proj(j) is issued 1-2 chunks ahead of the attention that needs it
        # (rope is pure-DVE now, so interleaving costs no DMA-ring blocking,
        # and fine-grained interleave keeps the vector queue from serializing
        # a bulk rope burst in front of the attention mask/divide ops).
        # outproj(j) is issued only after attn_pair(j+1, 0) so the PE queue
        # has attention matmuls while q-tile j's division chains drain.
        proj(0)
        proj(1)
        attn_pair(0, 0)
        proj(2)
        attn_pair(0, 1)
        proj(3)
        attn_pair(1, 0)
        for j in range(1, NCH):
            outproj(j - 1)
            attn_pair(j, 1)
            if j + 1 < NCH:
                attn_pair(j + 1, 0)
        outproj(NCH - 1)

    nc.compile()
    return nc


def _shard_inputs(x, Wq, Wk, Wv, Wo):
    # host-side pre-arrangement into SBUF layouts (contiguous DMA lines):
    # xT[p, j, kc, s] = x[b].T[kc*128+p, j*512+s]
    in_maps = []
    xb = []
    for b in range(B):
        xT = np.ascontiguousarray(x[b].T).astype(BF)          # [D, S]
        xb.append(np.ascontiguousarray(
            xT.reshape(8, 128, NCH, CW).transpose(1, 2, 0, 3)))
    for c in range(N_CORES):
        b, g = divmod(c, 4)
        wq = Wq[:, g * 256:(g + 1) * 256].astype(BF)          # [1024, 256]
        wkv = np.concatenate(
            [Wk[:, g * 64:(g + 1) * 64], Wv[:, g * 64:(g + 1) * 64]],
            axis=1).astype(BF)                                # [1024, 128]
        wo = Wo[g * 256:(g + 1) * 256, :].astype(BF)          # [256, 1024]
        in_maps.append({
            "xT": xb[b],
            "wq": np.ascontiguousarray(
                wq.reshape(8, 128, 256).transpose(1, 0, 2)),
            "wkv": np.ascontiguousarray(
                wkv.reshape(8, 128, 128).transpose(1, 0, 2)),
            "wo": np.ascontiguousarray(
                wo.reshape(2, 128, 1024).transpose(1, 0, 2)),
        })
    return in_maps


def kernel(x, Wq, Wk, Wv, Wo):
    x = np.asarray(x, dtype=np.float32)
    Wq = np.asarray(Wq, dtype=np.float32)
    Wk = np.asarray(Wk, dtype=np.float32)
    Wv = np.asarray(Wv, dtype=np.float32)
    Wo = np.asarray(Wo, dtype=np.float32)
    assert x.shape == (B, S, D), x.shape

    if "nc" not in _CACHE:
        _CACHE["nc"] = _build_kernel()
    nc = _CACHE["nc"]

    in_maps = _shard_inputs(x, Wq, Wk, Wv, Wo)
    res = run_bass_kernel_spmd(nc, in_maps, list(range(N_CORES)))

    out = np.zeros((B, S, D), dtype=np.float32)
    for c in range(N_CORES):
        out[c // 4] += np.asarray(res.results[c]["out"], dtype=np.float32)
    return out

